# revision 11
# baseline (speedup 1.0000x reference)
"""nn_GCN_31585189495371 — Trainium2 Bass kernel.

3-layer GCN over 256 independent 400-node graphs + per-graph readout.

Strategy (graph-level data parallel, 32 graphs/core on 8 cores):
  Host: convert the COO edge list into dense per-graph normalized adjacency
  (transposed: AT[src, dst]), lay out x transposed (xT[fin, node]) — pure
  format conversion so that all FLOPs (feature transforms, message passing,
  readout) run on-device as dense bf16 matmuls.
  Device per graph: xw^T = W1^T@xT ; per layer: node-part hw via
  "transposer" matmuls (lhsT = state^T slices), y^T = sum_c hw_c^T@AT_c,
  relu/bias epilogues on ACT; readout G += H3T[:,:,c]^T @ WcR_c; final Wl.
"""
import numpy as np
import ml_dtypes

G = 256
NPG = 400
N = G * NPG
FIN = 400
H = 64
NC = 8
GPC = G // NC  # 32

BF16 = ml_dtypes.bfloat16
_CACHE = {}


def _host_prep(x, edge_index, edge_weight, W1, b1, W2, b2, W3, b3, Wc, bc, Wl, bl):
    x = np.asarray(x, np.float32)
    ei = np.asarray(edge_index)
    ew = np.asarray(edge_weight, np.float32)

    src = ei[0].astype(np.int64)
    dst = ei[1].astype(np.int64)
    g_id = src // NPG
    sl = src - g_id * NPG
    dl = dst - g_id * NPG

    deg = np.bincount(dst, weights=ew.astype(np.float64), minlength=N) + 1.0
    dinv = (1.0 / np.sqrt(deg)).astype(np.float32)

    norm = dinv[src] * ew * dinv[dst]
    flat = g_id * (NPG * NPG) + sl * NPG + dl
    AT = np.bincount(flat, weights=norm.astype(np.float64),
                     minlength=G * NPG * NPG).astype(np.float32).reshape(G, NPG, NPG)
    idx = np.arange(NPG)
    AT[:, idx, idx] += (dinv * dinv).reshape(G, NPG)

    xg = x.reshape(G, NPG, FIN)
    xT = np.ascontiguousarray(xg.transpose(0, 2, 1))
    xT_feed = np.ascontiguousarray(
        xT.reshape(G, 4, 100, NPG).transpose(0, 2, 1, 3)).astype(BF16)

    AT_feed = np.ascontiguousarray(
        AT.reshape(G, 4, 100, NPG).transpose(0, 2, 1, 3)).astype(BF16)

    W1_feed = np.ascontiguousarray(
        np.asarray(W1, np.float32).reshape(4, 100, H).transpose(1, 0, 2)).astype(BF16)
    W2_feed = np.zeros((128, H), BF16)
    W2_feed[:64] = np.asarray(W2, np.float32).astype(BF16)
    W3_feed = np.zeros((128, H), BF16)
    W3_feed[:64] = np.asarray(W3, np.float32).astype(BF16)
    I_feed = np.zeros((128, 64), BF16)
    I_feed[:64] = np.eye(64, dtype=np.float32).astype(BF16)

    Wc = np.asarray(Wc, np.float32)
    WcR_feed = np.ascontiguousarray(
        Wc.reshape(NPG, H, H).transpose(1, 0, 2)).astype(BF16)

    Wl = np.asarray(Wl, np.float32)
    Wl_feed = Wl.astype(BF16)

    const_g = Wc.reshape(NPG, H, H).sum(0).T @ np.asarray(b3, np.float32) \
        + np.asarray(bc, np.float32)
    bl_final = (const_g @ Wl + np.asarray(bl, np.float32)).astype(np.float32)

    return dict(
        xT_feed=xT_feed, AT_feed=AT_feed, W1_feed=W1_feed, W2_feed=W2_feed,
        W3_feed=W3_feed, I_feed=I_feed, WcR_feed=WcR_feed, Wl_feed=Wl_feed,
        b1=np.asarray(b1, np.float32).reshape(H, 1),
        b2=np.asarray(b2, np.float32).reshape(H, 1),
        bl_final=bl_final.reshape(2, 1),
    )


def _build_bass():
    if "nc" in _CACHE:
        return _CACHE["nc"], _CACHE["tensors"]

    import concourse.mybir as mybir
    from concourse import bacc
    from concourse.tile import TileContext

    bf = mybir.dt.bfloat16
    f32 = mybir.dt.float32
    Relu = mybir.ActivationFunctionType.Relu

    nc = bacc.Bacc("TRN2", num_devices=NC)

    xt_in = nc.dram_tensor("xt", [GPC, 100, 4, NPG], bf, kind="ExternalInput").ap()
    at_in = nc.dram_tensor("at", [GPC, 100, 4, NPG], bf, kind="ExternalInput").ap()
    w1_in = nc.dram_tensor("w1", [100, 4, H], bf, kind="ExternalInput").ap()
    w2_in = nc.dram_tensor("w2", [128, H], bf, kind="ExternalInput").ap()
    w3_in = nc.dram_tensor("w3", [128, H], bf, kind="ExternalInput").ap()
    i_in = nc.dram_tensor("ident", [128, 64], bf, kind="ExternalInput").ap()
    wcr_in = nc.dram_tensor("wcr", [64, NPG, H], bf, kind="ExternalInput").ap()
    wl_in = nc.dram_tensor("wl", [64, 2], bf, kind="ExternalInput").ap()
    b1_in = nc.dram_tensor("b1", [H, 1], f32, kind="ExternalInput").ap()
    b2_in = nc.dram_tensor("b2", [H, 1], f32, kind="ExternalInput").ap()
    bl_in = nc.dram_tensor("bl", [2, 1], f32, kind="ExternalInput").ap()
    out_t = nc.dram_tensor("out", [2, GPC], f32, kind="ExternalOutput").ap()

    with TileContext(nc) as tc:
        with tc.tile_pool(name="consts", bufs=1) as cpool, \
             tc.tile_pool(name="feeds", bufs=3) as fpool, \
             tc.tile_pool(name="acc", bufs=3, space="PSUM") as apool, \
             tc.tile_pool(name="hwp", bufs=2, space="PSUM") as hpool, \
             tc.tile_pool(name="rops", bufs=1, space="PSUM") as rpool, \
             tc.tile_pool(name="hwsb", bufs=4) as spool:

            w1_t = cpool.tile([100, 4, H], bf, name="w1t")
            nc.sync.dma_start(w1_t, w1_in)
            w2_t = cpool.tile([128, H], bf, name="w2t")
            nc.sync.dma_start(w2_t, w2_in)
            w3_t = cpool.tile([128, H], bf, name="w3t")
            nc.sync.dma_start(w3_t, w3_in)
            i_t = cpool.tile([128, 64], bf, name="it")
            nc.sync.dma_start(i_t, i_in)
            wcr_t = cpool.tile([64, NPG, H], bf, name="wcrt")
            nc.sync.dma_start(wcr_t, wcr_in)
            wl_t = cpool.tile([64, 2], bf, name="wlt")
            nc.sync.dma_start(wl_t, wl_in)
            b1_t = cpool.tile([H, 1], f32, name="b1t")
            nc.sync.dma_start(b1_t, b1_in)
            b2_t = cpool.tile([H, 1], f32, name="b2t")
            nc.sync.dma_start(b2_t, b2_in)
            bl_t = cpool.tile([2, 1], f32, name="blt")
            nc.sync.dma_start(bl_t, bl_in)

            # persistent state buffers (upper halves must stay zero: the
            # transposer matmuls contract K=128 against zero weight rows)
            states = [cpool.tile([128, NPG], bf, name=f"st{i}") for i in range(4)]
            for st in states:
                nc.gpsimd.memset(st[:, :], 0.0)

            h3t = cpool.tile([64, GPC, NPG], bf, name="h3t")

            def transposer(state, w, tag):
                ps = hpool.tile([100, 4, H], mybir.dt.float32, tag="hwps")
                for s in range(4):
                    nc.tensor.matmul(ps[:, s, :], lhsT=state[:, s * 100:(s + 1) * 100],
                                     rhs=w[:, 0:H], start=True, stop=True)
                hw = spool.tile([100, 4, H], bf, tag="hwsb")
                nc.vector.tensor_copy(hw, ps)
                return hw

            def a_mult(hw, at, tag):
                ps = apool.tile([64, NPG], mybir.dt.float32, tag="accps")
                for c in range(4):
                    nc.tensor.matmul(ps, lhsT=hw[:, c, :], rhs=at[:, c, :],
                                     start=(c == 0), stop=(c == 3))
                return ps

            for g in range(GPC):
                xt = fpool.tile([100, 4, NPG], bf, tag="xt")
                nc.sync.dma_start(xt, xt_in[g])
                at = fpool.tile([100, 4, NPG], bf, tag="at")
                nc.sync.dma_start(at, at_in[g])

                s0 = states[(g % 2) * 2]
                s1 = states[(g % 2) * 2 + 1]

                # L1 feature transform: xw^T = sum_c W1_c^T @ xT_c
                ps = apool.tile([64, NPG], mybir.dt.float32, tag="accps")
                for c in range(4):
                    nc.tensor.matmul(ps, lhsT=w1_t[:, c, :], rhs=xt[:, c, :],
                                     start=(c == 0), stop=(c == 3))
                nc.vector.tensor_copy(s0[0:64, :], ps)

                hw = transposer(s0, i_t, "l1")
                ps = a_mult(hw, at, "a1")
                nc.scalar.activation(s1[0:64, :], ps, Relu, bias=b1_t, scale=1.0)

                hw = transposer(s1, w2_t, "l2")
                ps = a_mult(hw, at, "a2")
                nc.scalar.activation(s0[0:64, :], ps, Relu, bias=b2_t, scale=1.0)

                hw = transposer(s0, w3_t, "l3")
                ps = a_mult(hw, at, "a3")
                nc.vector.tensor_copy(h3t[:, g, :], ps)

            # readout: G[32, 64] = sum_c H3T[:, :, c]^T @ WcR[:, c, :]
            g_ps = rpool.tile([GPC, H], mybir.dt.float32, tag="gps")
            for c in range(NPG):
                nc.tensor.matmul(g_ps, lhsT=h3t[:, :, c], rhs=wcr_t[:, c, :],
                                 start=(c == 0), stop=(c == NPG - 1))
            g_sb = spool.tile([GPC, H], bf, tag="gsb")
            nc.vector.tensor_copy(g_sb, g_ps)
            # transpose [32, 64] -> [64, 32] via two 32x32 block transposes
            gt = spool.tile([64, GPC], bf, tag="gt")
            nc.vector.transpose(gt[0:32, :], g_sb[:, 0:32])
            nc.vector.transpose(gt[32:64, :], g_sb[:, 32:64])
            o_ps = rpool.tile([2, GPC], mybir.dt.float32, tag="ops")
            nc.tensor.matmul(o_ps, lhsT=wl_t, rhs=gt, start=True, stop=True)
            o_sb = spool.tile([2, GPC], mybir.dt.float32, tag="osb")
            nc.vector.tensor_scalar_add(o_sb, o_ps, bl_t)
            nc.sync.dma_start(out_t, o_sb)

    nc.finalize()
    _CACHE["nc"] = nc
    _CACHE["tensors"] = None
    return nc, None


def kernel(x, edge_index, edge_weight, W1, b1, W2, b2, W3, b3, Wc, bc, Wl, bl):
    from concourse.bass_utils import run_bass_kernel_spmd

    P = _host_prep(x, edge_index, edge_weight, W1, b1, W2, b2, W3, b3,
                   Wc, bc, Wl, bl)
    nc, _ = _build_bass()

    res = run_bass_kernel_spmd(nc, _in_maps(P), core_ids=list(range(NC)))
    full = np.zeros((G, 2), np.float32)
    for c in range(NC):
        full[c * GPC:(c + 1) * GPC] = np.asarray(res.results[c]["out"]).T
    return full


def _in_maps(P):
    maps = []
    for c in range(NC):
        sl = slice(c * GPC, (c + 1) * GPC)
        maps.append({
            "xt": P["xT_feed"][sl], "at": P["AT_feed"][sl],
            "w1": P["W1_feed"], "w2": P["W2_feed"], "w3": P["W3_feed"],
            "ident": P["I_feed"], "wcr": P["WcR_feed"], "wl": P["Wl_feed"],
            "b1": P["b1"], "b2": P["b2"], "bl": P["bl_final"],
        })
    return maps


def profile_hw(inputs, trace_dir=None):
    """Re-run with NTFF tracing; returns HW exec_time_ns (slowest core)."""
    from concourse.bass_utils import run_bass_kernel_spmd

    P = _host_prep(**inputs)
    nc, _ = _build_bass()
    res = run_bass_kernel_spmd(nc, _in_maps(P), core_ids=list(range(NC)),
                               trace=True, tmpdir=trace_dir)
    if res.instructions_and_trace is not None:
        print("trace:", res.instructions_and_trace[1])
    print("profile_json:", res.profile_json)
    return res.exec_time_ns
